# revision 27
# baseline (speedup 1.0000x reference)
"""Quantized Linear (int8-valued GEMM + zero-point corrections) on 8 TRN2 cores.

y = (a @ w).f32 * a_s * w_s
  + (a.f32 * a_s).rowsum * w_o          (per-row correction)
  + a_o * (w.f32 * w_s).colsum          (per-col correction)
  + K * a_o * w_o                       (constant)

Sharding: 2D tensor-parallel grid, 4 shards over M (rows of a) x 2 shards
over N (cols of w).  Each core computes a [1024, 2048] slice of the output.

Device kernel per core (values 0..126 are exact in bf16):
  - warm-up: throwaway narrow matmuls on memset tiles keep the PE busy from
    t~1us so the clock ramp (HAM warm-up on real HW) completes inside the
    startup-DMA shadow instead of on real matmuls
  - startup: a (kt-major layout) and w stream in as interleaved 2-k-tile
    slices; chunk 0 runs k-outer over a 5-m-tile then a 3-m-tile group so
    the PE consumes each freshly-landed w k-plane 5x and stays just behind
    the serialized DMA stream; later chunks are fully prefetched and run
    m-outer with a 6-deep PSUM rotation
  - main GEMM in bf16 with fp32 PSUM accumulation (exact per-matmul: 128-dot
    of products <= 16129*128 < 2^24)
  - row-sums of a via piggybacked N=1 matmuls sharing the stationary operand;
    rowbias scalars on gpsimd (keeps DVE free for colsum trees)
  - col-sums of w via DVE log-halving over k-tiles, a 1/16-scaled fp16
    convert (max |colsum|/16 < 2^15, rounding error ~0.1 abs vs ~11 budget),
    then an fp16 matmul against a (16*a_o*w_s)-filled [128,128] matrix
    (partition-reduce + broadcast + scale in one op, 1 cycle/col vs 4 for
    fp32)
  - epilogue: out = psum * (a_s*w_s) + rowbias (scalar engine) then
    += colsum_bcast (DVE), DMA out
  - the last m-tile of the last chunk is split into 4x128-col pieces (their
    colsum term folded into PSUM via an fp16 matmul mid-accumulation) so the
    final tail after the last matmul is last-matmul -> ACT -> DMA only

Input scalars are baked into the program as immediates (compiled per call).
"""

import sys

for _p in ("/opt/trn_rl_repo",):
    if _p not in sys.path:
        sys.path.insert(0, _p)

import numpy as np
import ml_dtypes

BF16 = ml_dtypes.bfloat16

P = 128
M, K, N = 4096, 4096, 4096
GM, GN = 4, 2  # shard grid: 4 over M, 2 over N
MC, NC = M // GM, N // GN  # per-core output slice: 1024 x 2048
CW = 512  # n-chunk width (one PSUM bank)
N_CORES = GM * GN

N_WARMUP = 57  # throwaway ramp matmuls (cover ~1.1us .. ~4.2us)
WARM_W = 64  # their moving width
TAIL_WS = [128, 128, 128, 128]  # last m-tile piece widths
G0 = 5  # chunk-0 m-group size
KCUTS = [0, 2, 4, 6, 8, 10, 12, 14, 16, 20, 24, 28, 32]  # chunk-0 k-slices

_cached = {}


def _build_program(ko, mo, nch, cw, s1, c1, c2, beta):
    """Build the single-core Bass/Tile program (SPMD: same program, per-core data)."""
    import concourse.bacc as bacc
    import concourse.mybir as mybir
    import concourse.tile as tile

    f32 = mybir.dt.float32
    f16 = mybir.dt.float16
    bf16 = mybir.dt.bfloat16
    ADD = mybir.AluOpType.add
    MULT = mybir.AluOpType.mult
    IDENT = mybir.ActivationFunctionType.Identity

    ncl = nch * cw

    nc = bacc.Bacc(None, target_bir_lowering=False)
    # lhsT is kt-major: [p, kt, mt, c] = a[mt*128 + c, kt*128 + p] so that a
    # per-kt slice over a group of m-tiles is contiguous for the DMA.
    lhsT_d = nc.dram_tensor("lhsT", [P, ko, mo, P], bf16, kind="ExternalInput")
    rhs_d = nc.dram_tensor("rhs", [P, ko, ncl], bf16, kind="ExternalInput")
    out_d = nc.dram_tensor("out", [P, mo, ncl], f32, kind="ExternalOutput")

    with tile.TileContext(nc) as tc:
        with (
            tc.tile_pool(name="const", bufs=1) as constp,
            tc.tile_pool(name="lhs", bufs=1) as lhsp,
            tc.tile_pool(name="wpool", bufs=2) as wp,
            tc.tile_pool(name="cs1", bufs=1) as cs1p,
            tc.tile_pool(name="cs2", bufs=1) as cs2p,
            tc.tile_pool(name="cs3", bufs=2) as cs3p,
            tc.tile_pool(name="colbc", bufs=2) as colbcp,
            tc.tile_pool(name="stage", bufs=4) as stagep,
            tc.tile_pool(name="stail", bufs=2) as stailp,
            tc.tile_pool(name="pmain", bufs=6, space="PSUM") as pmain,
            tc.tile_pool(name="pcol", bufs=1, space="PSUM") as pcol,
            tc.tile_pool(name="prs", bufs=1, space="PSUM") as prs,
        ):
            # ---- warm-up tiles first so their memsets land at t~0
            wu_lhs = constp.tile([P, P], bf16)
            nc.gpsimd.memset(wu_lhs[:], 0.0)
            wu_mov = constp.tile([P, WARM_W], bf16)
            nc.gpsimd.memset(wu_mov[:], 0.0)

            ones_mov = constp.tile([P, 1], bf16)
            nc.vector.memset(ones_mov[:], 1.0)
            # fp16 colsum matmul stationary: (16*a_o*w_s) * ones[128,128];
            # (bw16.T @ cs16)[m, n] = 16*a_o*w_s * sum_p cs16[p, n]
            bw16 = constp.tile([P, P], f16)
            nc.vector.memset(bw16[:], beta * 16.0)
            # tail fold stationary: pre-scale version so ACT's *s1 restores it
            bw16b = constp.tile([P, P], f16)
            nc.vector.memset(bw16b[:], beta / s1 * 16.0)
            c2_t = constp.tile([P, 1], f32)
            nc.vector.memset(c2_t[:], c2)

            # preload the ACT function table off the critical path
            act_warm = constp.tile([P, 1], f32)
            nc.scalar.activation(act_warm[:], c2_t[:], IDENT, scale=1.0)

            rowbias = constp.tile([P, mo], f32)
            rs_ps = prs.tile([P, mo], f32)

            lhsT_sb = lhsp.tile([P, ko, mo, P], bf16)

            # ---- warm-up matmuls: keep PE busy from ~1us so the p-state
            # ramp finishes while the first data DMAs are in flight.  The
            # scratch PSUM tile comes from the pmain pool; a later chunk-0
            # group tile reuses its bank (WAR dep serializes on PE anyway).
            wu_ps = pmain.tile([P, cw], f32, tag="pmain", name="wu_ps")
            for i in range(N_WARMUP):
                nc.tensor.matmul(
                    wu_ps[:, 0:WARM_W], wu_lhs[:], wu_mov[:], start=True, stop=True
                )

            # ---- startup DMAs (SP queue order == HWDGE/DMA service order):
            # interleaved (a, w) k-slices for chunk 0 + group 0, then group-1
            # a in 4-kt pieces, then chunk 1.
            wt0 = wp.tile([P, ko, cw], bf16, tag="wchunk", name="wt0")
            for k0, k1 in zip(KCUTS[:-1], KCUTS[1:]):
                nc.sync.dma_start(
                    out=lhsT_sb[:, k0:k1, 0:G0], in_=lhsT_d[:, k0:k1, 0:G0]
                )
                nc.sync.dma_start(out=wt0[:, k0:k1, :], in_=rhs_d[:, k0:k1, 0:cw])
            for k0 in range(0, ko, 4):
                nc.sync.dma_start(
                    out=lhsT_sb[:, k0 : k0 + 4, G0:mo],
                    in_=lhsT_d[:, k0 : k0 + 4, G0:mo],
                )

            def load_chunk(ci):
                wt = wp.tile([P, ko, cw], bf16, tag="wchunk", name=f"wt{ci}")
                dchunk = max(1, ko // 4)
                for i in range(0, ko, dchunk):
                    nc.sync.dma_start(
                        out=wt[:, i : i + dchunk, :],
                        in_=rhs_d[:, i : i + dchunk, ci * cw : (ci + 1) * cw],
                    )
                return wt

            wt1 = load_chunk(1)
            cs16_by_ci = {}

            def colsum_bcast(ci, wt):
                # reduce over k-tiles: one exact bf16 level (sums <= 252), then f32
                h = ko // 2
                s1t = cs1p.tile([P, h, cw], bf16, tag="cs_bf", name=f"cs1_{ci}")
                nc.vector.tensor_add(s1t[:], wt[:, 0:h, :], wt[:, h : 2 * h, :])
                h //= 2
                s2t = cs2p.tile([P, max(h, 1), cw], f32, tag="cs_f32", name=f"cs2_{ci}")
                if h >= 1:
                    nc.vector.tensor_add(s2t[:, 0:h], s1t[:, 0:h, :], s1t[:, h : 2 * h, :])
                else:
                    nc.vector.tensor_copy(out=s2t[:, 0:1], in_=s1t[:, 0:1, :])
                while h > 1:
                    h //= 2
                    nc.vector.tensor_add(s2t[:, 0:h], s2t[:, 0:h], s2t[:, h : 2 * h])
                # 1/16-scale to fp16 (scalar engine), then fp16 matmul:
                # partition-reduce + broadcast + (16*a_o*w_s) scale in one shot
                cs16 = cs3p.tile([P, cw], f16, tag="cs16", name=f"cs16_{ci}")
                nc.scalar.activation(cs16[:], s2t[:, 0, :], IDENT, scale=1.0 / 16.0)
                cs16_by_ci[ci] = cs16
                pc = pcol.tile([P, cw], f32, tag="pcol", name=f"pc{ci}")
                nc.tensor.matmul(pc[:], bw16[:], cs16[:], start=True, stop=True)
                col_sb = colbcp.tile([P, cw], f32, tag="colbc", name=f"colsb{ci}")
                nc.scalar.copy(out=col_sb[:], in_=pc[:])
                return col_sb

            col_sb = colsum_bcast(0, wt0)

            def make_rowbias_group(g0, g1):
                # rowbias = rowsum * (a_s*w_o) + K*a_o*w_o in ONE scalar-
                # engine op (c1 immediate scale, c2 per-partition bias).
                # Single read frees rs_ps fast (coarse WAR tracking would
                # otherwise park the next group's rowsum matmul starts), and
                # the epilogue ACTs queue right behind it with no sem hop.
                nc.scalar.activation(
                    rowbias[:, g0:g1], rs_ps[:, g0:g1], IDENT,
                    bias=c2_t[:], scale=c1,
                )

            def epilogue(ps, mi, ci, c0, w, pool, add_col=True):
                # st = ps*s1 + rowbias   (scalar engine, per-partition bias)
                st = pool.tile(
                    [P, w], f32, tag=f"st{w}", name=f"st_{ci}_{mi}_{c0}"
                )
                nc.scalar.activation(
                    st[:], ps, IDENT, bias=rowbias[:, mi : mi + 1], scale=s1
                )
                if add_col:
                    nc.vector.tensor_add(st[:], st[:], col_sb[:, c0 : c0 + w])
                nc.sync.dma_start(
                    out=out_d[:, mi, ci * cw + c0 : ci * cw + c0 + w], in_=st[:]
                )

            # ---- chunk 0: k-outer over two 4-m-tile groups
            wt = wt0
            for g0 in range(0, mo, G0):
                g1 = min(g0 + G0, mo)
                ps_g = [
                    pmain.tile([P, cw], f32, tag="pmain", name=f"ps_c0_m{mi}")
                    for mi in range(g0, g1)
                ]
                for kt in range(ko):
                    for gi, mi in enumerate(range(g0, g1)):
                        lhs_ap = lhsT_sb[:, kt, mi, :]
                        nc.tensor.matmul(
                            ps_g[gi][:],
                            lhs_ap,
                            wt[:, kt, :],
                            start=(kt == 0),
                            stop=(kt == ko - 1),
                        )
                        # same stationary operand as the main matmul above.
                        # start=True only on the group's FIRST rowsum matmul:
                        # on HW, start clears the whole PSUM bank, so giving
                        # every column's group its own start would wipe the
                        # kt=0 writes of the columns started earlier.  The
                        # later columns' kt=0 matmuls land on cleared /
                        # never-written elements where accumulate==overwrite.
                        nc.tensor.matmul(
                            rs_ps[:, mi : mi + 1],
                            lhs_ap,
                            ones_mov[:],
                            start=(kt == 0 and gi == 0),
                            stop=(kt == ko - 1),
                        )
                make_rowbias_group(g0, g1)
                for gi, mi in enumerate(range(g0, g1)):
                    epilogue(ps_g[gi][:], mi, 0, 0, cw, stagep)

            # ---- chunks 1..nch-1: fully prefetched, m-outer
            wt_next = wt1
            for ci in range(1, nch):
                wt = wt_next
                col_sb = colsum_bcast(ci, wt)
                last_chunk = ci == nch - 1
                m_last = mo - 1 if last_chunk else mo
                for mi in range(m_last):
                    if mi == 1 and ci + 1 < nch:
                        wt_next = load_chunk(ci + 1)
                    ps = pmain.tile(
                        [P, cw], f32, tag="pmain", name=f"ps_{ci}_{mi}"
                    )
                    for kt in range(ko):
                        nc.tensor.matmul(
                            ps[:],
                            lhsT_sb[:, kt, mi, :],
                            wt[:, kt, :],
                            start=(kt == 0),
                            stop=(kt == ko - 1),
                        )
                    epilogue(ps[:], mi, ci, 0, cw, stagep)
                if last_chunk:
                    # final m-tile in narrow pieces => small epilogue tail
                    mi = mo - 1
                    cs16_last = cs16_by_ci[ci]
                    c0 = 0
                    for tw in TAIL_WS:
                        pt = pmain.tile(
                            [P, tw], f32, tag="pmain", name=f"pt{c0}"
                        )
                        for kt in range(ko):
                            nc.tensor.matmul(
                                pt[:],
                                lhsT_sb[:, kt, mi, :],
                                wt[:, kt, c0 : c0 + tw],
                                start=(kt == 0),
                                stop=(kt == ko - 1),
                            )
                            if kt == 0:
                                # fold the colsum term into the accumulation:
                                # += (a_o/a_s)*colsum; ACT's *s1 makes it
                                # a_o*w_s*colsum.  Skips the DVE add in the
                                # final epilogue tail.
                                nc.tensor.matmul(
                                    pt[:],
                                    bw16b[:],
                                    cs16_last[:, c0 : c0 + tw],
                                    start=False,
                                    stop=False,
                                )
                        epilogue(pt[:], mi, ci, c0, tw, stailp, add_col=False)
                        c0 += tw
    nc.compile()
    return nc


def _get_program(ko, mo, nch, cw, s1, c1, c2, beta):
    key = (ko, mo, nch, cw, float(s1), float(c1), float(c2), float(beta))
    if key not in _cached:
        _cached[key] = _build_program(ko, mo, nch, cw, s1, c1, c2, beta)
    return _cached[key]


def _scalars(a_s, a_o, w_s, w_o, k):
    a_s_f = np.float32(np.asarray(a_s).reshape(-1)[0])
    a_o_f = np.float32(np.asarray(a_o).reshape(-1)[0])
    w_s_f = np.float32(np.asarray(w_s).reshape(-1)[0])
    w_o_f = np.float32(np.asarray(w_o).reshape(-1)[0])
    s1 = float(a_s_f * w_s_f)
    c1 = float(a_s_f * w_o_f)
    c2 = float(np.float32(k) * a_o_f * w_o_f)
    bw = float(a_o_f * w_s_f)  # colsum scale (== beta*s1, computed directly)
    return s1, c1, c2, bw


def _make_in_maps(a, w, gm, gn):
    m, k = a.shape
    _, n = w.shape
    mc, ncl = m // gm, n // gn
    ko = k // P

    a_bf = a.astype(BF16)
    w_bf = w.astype(BF16)

    in_maps = []
    for mi in range(gm):
        # aT slice tiled kt-major to [P, KO, MO, P]:
        # [p, kt, mt, c] = a[mi*mc + mt*128 + c, kt*128 + p]
        a_sl = a_bf[mi * mc : (mi + 1) * mc, :]  # [mc, k]
        lhsT = np.ascontiguousarray(
            a_sl.T.reshape(ko, P, mc // P, P).transpose(1, 0, 2, 3)
        )
        for nj in range(gn):
            w_sl = w_bf[:, nj * ncl : (nj + 1) * ncl]  # [k, ncl]
            rhs = np.ascontiguousarray(w_sl.reshape(ko, P, ncl).transpose(1, 0, 2))
            in_maps.append({"lhsT": lhsT, "rhs": rhs})
    return in_maps


def _run(a, a_s, a_o, w, w_s, w_o, gm=GM, gn=GN, cw=CW, trace=False):
    from concourse.bass_utils import run_bass_kernel_spmd

    m, k = a.shape
    _, n = w.shape
    mc, ncl = m // gm, n // gn
    s1, c1, c2, beta = _scalars(a_s, a_o, w_s, w_o, k)
    nc = _get_program(k // P, mc // P, ncl // cw, cw, s1, c1, c2, beta)
    in_maps = _make_in_maps(a, w, gm, gn)
    res = run_bass_kernel_spmd(nc, in_maps, list(range(gm * gn)), trace=trace)

    out = np.empty((m, n), dtype=np.float32)
    for mi in range(gm):
        for nj in range(gn):
            r = res.results[mi * gn + nj]["out"]  # [P, MO, NCL]
            out[mi * mc : (mi + 1) * mc, nj * ncl : (nj + 1) * ncl] = (
                r.transpose(1, 0, 2).reshape(mc, ncl)
            )
    return out, res


def kernel(a, a_s, a_o, w, w_s, w_o):
    out, _ = _run(
        np.asarray(a), np.asarray(a_s), np.asarray(a_o),
        np.asarray(w), np.asarray(w_s), np.asarray(w_o),
    )
    return out
